# revision 31
# baseline (speedup 1.0000x reference)
"""BatchGRU Trainium2 kernel.

Bidirectional GRU over 256 ragged graph sequences (L=128, H=1024),
data-parallel over graphs x direction on 8 NeuronCores:
  cores 0-3: forward direction, 64 graph-ranks each
  cores 4-7: backward direction, 64 graph-ranks each

Two programs (one per direction group) run concurrently on disjoint core
groups. Graphs are sorted by atom count and dealt 4-way so all cores share
one descending per-rank capacity profile (caps, multiples of 8) -> every
DMA access pattern is identical across the cores of a group (SPMD).

Per core:
  A) projection over the COMPACT slot layout (sum(caps) rows, not 64*128):
     PE-transpose atoms -> h^T chunks; h0 via free-dim reduce_max (pads are
     -1e30); msgs^T = Relu(h^T + bias) fused on ACT; matmul msgs^T @ w_ih^T
     with biases folded in as a rank-1 ones-row matmul; xg stored time-major
     per rank (bwd stores end-aligned: row q -> step q + 128 - cap_r).
  B) recurrence (fwd: caps[0] steps, bwd: 128): per step one prefix DMA of
     real xg rows + one const DMA for padded ranks; hg = h @ w_hh^T on PE
     (fp32r, W as moving operand; xg and b_hh_n injected into PSUM via
     identity / ones rank-1 matmuls), gates on ACT/DVE/GPSIMD, 8 small PE
     transposes produce the next stationary h^T; h stored time-major.
"""

import numpy as np
import ml_dtypes
_bf16 = ml_dtypes.bfloat16

H = 1024
G = 256
L = 128
N_CORES = 8
GPC = 64  # graph-ranks per core (one direction)
KC = 8  # H // 128 contraction chunks
H3 = 3 * H
PAD_VAL = -1e30

_PROG_CACHE = {}
_LAST_IN_MAPS = None


def _build_program(direction, caps, n_steps, dsub='dve', xnbufs=3, gwbufs=1, htsplit=False):
    import concourse.mybir as mybir
    import concourse.tile as tile
    from concourse import bacc
    from concourse.masks import make_identity

    F32 = mybir.dt.float32
    F32R = mybir.dt.float32r
    BF16 = mybir.dt.bfloat16
    AF = mybir.ActivationFunctionType
    OP = mybir.AluOpType
    AX = mybir.AxisListType

    caps = list(caps)
    base = np.concatenate([[0], np.cumsum(caps)]).astype(int)
    S = int(base[-1])
    SP = ((S + 127) // 128) * 128
    NT = SP // 128
    # per-step prefix of ranks whose xg row at that step is real (stored)
    if direction == "f":
        kreal = [sum(1 for c in caps if c > t) for t in range(n_steps)]
    else:
        kreal = [sum(1 for c in caps if c >= L - t) for t in range(n_steps)]
    shift = [0 if direction == "f" else L - c for c in caps]

    nc = bacc.Bacc("TRN2", target_bir_lowering=False, debug=False)

    h_cmp = nc.dram_tensor("h_cmp", [SP, H], BF16, kind="ExternalInput").ap()
    w_ihT = nc.dram_tensor("w_ihT", [KC, 128, H3], BF16, kind="ExternalInput").ap()
    w_hhT = nc.dram_tensor("w_hhT", [KC, 128, H3], BF16, kind="ExternalInput").ap()
    biasT = nc.dram_tensor("biasT", [128, KC], F32, kind="ExternalInput").ap()
    pbias = nc.dram_tensor("pbias", [1, H3], BF16, kind="ExternalInput").ap()
    padgr = nc.dram_tensor("padgr", [GPC, H3], BF16, kind="ExternalInput").ap()
    bhh_n = nc.dram_tensor("bhh_n", [1, H], BF16, kind="ExternalInput").ap()
    ones_d = nc.dram_tensor("ones_d", [1, 128], BF16, kind="ExternalInput").ap()
    id64_d = nc.dram_tensor("id64_d", [GPC, GPC], BF16, kind="ExternalInput").ap()
    out_stage = nc.dram_tensor("out_stage", [L, GPC, H], BF16, kind="ExternalOutput").ap()
    xg_tm = nc.dram_tensor("xg_tm", [L, GPC, H3], BF16).ap()  # scratch

    # rank segments per 128-row tile: list of (tile, row0_in_tile, nrows,
    # rank, q0) covering [base_r, base_r + cap_r)
    segs_by_tile = [[] for _ in range(NT)]
    for r in range(GPC):
        q = 0
        while q < caps[r]:
            row = base[r] + q
            ti = row // 128
            take = min(caps[r] - q, (ti + 1) * 128 - row)
            segs_by_tile[ti].append((row - ti * 128, take, r, q))
            q += take

    with tile.TileContext(nc) as tc:
        with (
            tc.tile_pool(name="wpool", bufs=1) as wpool,
            tc.tile_pool(name="consts", bufs=1) as consts,
            tc.tile_pool(name="state", bufs=2) as state,
        ):
            ident = consts.tile([128, 128], BF16)
            make_identity(nc, ident)
            ones_r = consts.tile([1, 128], BF16)
            nc.sync.dma_start(out=ones_r, in_=ones_d)
            biasT_s = consts.tile([128, KC], F32)
            nc.sync.dma_start(out=biasT_s, in_=biasT)
            pbias_bc = consts.tile([128, H3], BF16)
            nc.sync.dma_start(out=pbias_bc, in_=pbias.to_broadcast((128, H3)))
            bhh_n_s = consts.tile([1, H], BF16)
            nc.sync.dma_start(out=bhh_n_s, in_=bhh_n)
            id64_s = consts.tile([GPC, GPC], BF16)
            nc.sync.dma_start(out=id64_s, in_=id64_d)

            # ---------------- Phase A: projection (compact layout) --------
            w_ih_s = wpool.tile([128, KC, H3], BF16, tag="w")
            for k in range(KC):
                nc.sync.dma_start(out=w_ih_s[:, k, :], in_=w_ihT[k])

            h0T_f = consts.tile([128, KC, GPC], BF16)  # raw h0^T
            h0tmp = consts.tile([128, 1], BF16)

            with (
                tc.tile_pool(name="pa", bufs=2, space="PSUM") as pa,
                tc.tile_pool(name="pt", bufs=2, space="PSUM") as pt,
                tc.tile_pool(name="aw", bufs=2) as aw,
            ):
                for ti in range(NT):
                    if not segs_by_tile[ti]:
                        continue
                    hp = aw.tile([128, H], BF16, tag="hp")
                    nc.scalar.dma_start(out=hp, in_=h_cmp[ti * 128:(ti + 1) * 128, :])
                    msgsT = aw.tile([128, KC, 128], BF16, tag="msgsT")
                    rawT = aw.tile([128, KC, 128], BF16, tag="rawT")
                    for c in range(KC):
                        pst = pt.tile([128, 128], BF16, tag="tp")
                        nc.tensor.transpose(pst, hp[:, c * 128:(c + 1) * 128], ident)
                        # drain pst fast (2 readers only) so PE transposes
                        # aren't stalled behind the per-rank h0 reduces
                        nc.vector.tensor_copy(rawT[:, c, :], pst)
                        nc.scalar.activation(
                            msgsT[:, c, :], pst, AF.Relu, bias=biasT_s[:, c:c + 1])
                        for (r0, nr, rank, q0) in segs_by_tile[ti]:
                            if q0 == 0:
                                nc.vector.tensor_reduce(
                                    out=h0T_f[:, c, rank:rank + 1],
                                    in_=rawT[:, c, r0:r0 + nr], axis=AX.X, op=OP.max)
                            else:
                                nc.vector.tensor_reduce(
                                    out=h0tmp, in_=rawT[:, c, r0:r0 + nr],
                                    axis=AX.X, op=OP.max)
                                nc.vector.tensor_tensor(
                                    h0T_f[:, c, rank:rank + 1],
                                    h0T_f[:, c, rank:rank + 1], h0tmp, op=OP.max)
                    for half in range(2):
                        pm = pa.tile([128, 1536], F32, tag="pm")
                        for b in range(3):
                            col0 = half * 1536 + b * 512
                            for k in range(KC):
                                nc.tensor.matmul(
                                    pm[:, b * 512:(b + 1) * 512], msgsT[:, k, :],
                                    w_ih_s[:, k, col0:col0 + 512],
                                    start=(k == 0), stop=(k == KC - 1))
                        xgs = aw.tile([128, 1536], BF16, tag="xgs")
                        nc.vector.tensor_tensor(
                            xgs, pm,
                            pbias_bc[:, half * 1536:(half + 1) * 1536], op=OP.add)
                        for si, (r0, nr, rank, q0) in enumerate(segs_by_tile[ti]):
                            t0 = q0 + shift[rank]
                            eng = nc.sync if si % 2 == 0 else nc.scalar
                            eng.dma_start(
                                out=xg_tm[t0:t0 + nr, rank,
                                          half * 1536:(half + 1) * 1536],
                                in_=xgs[r0:r0 + nr, :])

            # ---------------- Phase B: recurrence ----------------
            w_hh_s = wpool.tile([128, KC, H3], BF16, tag="w")
            for k in range(KC):
                nc.sync.dma_start(out=w_hh_s[:, k, :], in_=w_hhT[k])

            # initial state: hT (f32r) and h (natural layout)
            if htsplit:
                hT_lo = state.tile([128, KC // 2, GPC], BF16, tag="hTl")
                hT_hi = state.tile([128, KC // 2, GPC], BF16, tag="hTh")
                nc.scalar.copy(hT_lo, h0T_f[:, :KC // 2, :])
                nc.scalar.copy(hT_hi, h0T_f[:, KC // 2:, :])
                hT_pair = (hT_lo, hT_hi)
            else:
                hT = state.tile([128, KC, GPC], BF16, tag="hT")
                nc.scalar.copy(hT, h0T_f)
            h_s = state.tile([64, H], BF16, tag="h")
            with tc.tile_pool(name="pi", bufs=2, space="PSUM") as pti:
                for c in range(KC):
                    pst = pti.tile([GPC, 128], BF16, tag="tp")
                    nc.tensor.transpose(pst, h0T_f[:, c, :], ident)
                    nc.scalar.copy(h_s[:, c * 128:(c + 1) * 128], pst)

            with (
                tc.tile_pool(name="pb", bufs=1, space="PSUM") as pb,
                tc.tile_pool(name="ptr", bufs=2, space="PSUM") as ptrp,
                tc.tile_pool(name="gw", bufs=gwbufs) as gw,
                tc.tile_pool(name="sw", bufs=1) as sw,
                tc.tile_pool(name="xn", bufs=xnbufs) as xnp,
            ):
                for t in range(n_steps):
                    xg_s = xnp.tile([64, H3], BF16, tag="xg")
                    k_t = kreal[t]
                    if k_t > 0:
                        nc.sync.dma_start(out=xg_s[:k_t, :], in_=xg_tm[t, :k_t, :])
                    if k_t < GPC:
                        nc.sync.dma_start(out=xg_s[k_t:, :], in_=padgr[k_t:, :])

                    pr = pb.tile([64, H], F32, tag="pr")
                    pn = pb.tile([64, H], F32, tag="pn")
                    pz = pb.tile([64, H], F32, tag="pz")

                    r = gw.tile([64, H], F32, tag="r")
                    u = gw.tile([64, H], F32, tag="u")
                    n_g = gw.tile([64, H], F32, tag="n")
                    z = gw.tile([64, H], F32, tag="z")
                    d = sw.tile([64, H], F32, tag="d")
                    e = sw.tile([64, H], F32, tag="e")
                    h_new = state.tile([64, H], BF16, tag="h")
                    if htsplit:
                        hTn_lo = state.tile([128, KC // 2, GPC], BF16, tag="hTl")
                        hTn_hi = state.tile([128, KC // 2, GPC], BF16, tag="hTh")
                    else:
                        hT_next = state.tile([128, KC, GPC], BF16, tag="hT")
                    ptr_t = ptrp.tile([128, 512], BF16, tag="ptr")

                    # matmuls: per gate, inject xg/bhh per half (f32r,
                    # 512-wide) then 8 bf16 1024-wide W matmuls
                    for gate in ("r", "n", "z"):
                        tile_, wcol = {"r": (pr, 0), "z": (pz, 1024), "n": (pn, 2048)}[gate]
                        for hf in range(2):
                            sl = slice(hf * 512, hf * 512 + 512)
                            if gate == "n":
                                nc.tensor.matmul(tile_[:, sl], ones_r[:, :GPC],
                                                 bhh_n_s[:, sl], start=True, stop=False)
                            else:
                                nc.tensor.matmul(
                                    tile_[:, sl], id64_s,
                                    xg_s[:, wcol + hf * 512:wcol + hf * 512 + 512],
                                    start=True, stop=False)
                        for hf in range(2):
                            sl = slice(hf * 512, hf * 512 + 512)
                            for k in range(KC):
                                nc.tensor.matmul(
                                    tile_[:, sl], hT[:, k, :],
                                    w_hh_s[:, k, wcol + hf * 512:wcol + hf * 512 + 512],
                                    start=False, stop=(k == KC - 1))

                    for hf in range(2):
                        sl = slice(hf * 512, hf * 512 + 512)

                        # gate chain for this half
                        nc.scalar.activation(r[:, sl], pr[:, sl], AF.Sigmoid)
                        nc.vector.tensor_tensor(u[:, sl], r[:, sl], pn[:, sl], op=OP.mult)
                        nc.vector.tensor_tensor(
                            u[:, sl], u[:, sl],
                            xg_s[:, 2048 + hf * 512:2048 + hf * 512 + 512],
                            op=OP.add)
                        nc.scalar.activation(n_g[:, sl], u[:, sl], AF.Tanh)
                        (nc.gpsimd if dsub == 'gp' else nc.vector).tensor_sub(d[:, sl], h_s[:, sl], n_g[:, sl])
                        nc.scalar.activation(z[:, sl], pz[:, sl], AF.Sigmoid)
                        for q in range(2):
                            qs = slice(hf * 512 + q * 256, hf * 512 + q * 256 + 256)
                            nc.vector.tensor_tensor(e[:, qs], z[:, qs], d[:, qs], op=OP.mult)
                            nc.vector.tensor_tensor(h_new[:, qs], n_g[:, qs], e[:, qs], op=OP.add)
                            if t < n_steps - 1:
                                c0 = 4 * hf + q * 2
                                for c in (c0, c0 + 1):
                                    nc.tensor.transpose(
                                        ptr_t[:, c * 64:(c + 1) * 64],
                                        h_new[:, c * 128:(c + 1) * 128],
                                        ident[:64, :64])
                                cp = nc.vector.tensor_copy if hf else nc.scalar.copy
                                if htsplit:
                                    dst = (hTn_lo if c0 < 4 else hTn_hi)[:, c0 % 4:c0 % 4 + 2, :]
                                else:
                                    dst = hT_next[:, c0:c0 + 2, :]
                                cp(dst, ptr_t[:, c0 * 64:(c0 + 2) * 64])

                    nout = max(k_t, 1)
                    nc.sync.dma_start(out=out_stage[t, :nout, :], in_=h_new[:nout, :])
                    if t < n_steps - 1:
                        if htsplit:
                            hT_pair = (hTn_lo, hTn_hi)
                        else:
                            hT = hT_next
                    h_s = h_new

    nc.compile()
    return nc, SP


def _get_programs(caps):
    key = tuple(caps)
    if key not in _PROG_CACHE:
        nf, SPf = _build_program("f", caps, int(caps[0]), dsub="gp")
        nb, SPb = _build_program("b", caps, L, dsub="gp")
        _PROG_CACHE[key] = (nf, nb, SPf)
    return _PROG_CACHE[key]


def _make_runner(nc, n_group, dev_offset):
    import jax
    from jax.sharding import Mesh, PartitionSpec
    from jax.experimental.shard_map import shard_map
    import concourse.mybir as mybir
    from concourse.bass2jax import (_bass_exec_p, install_neuronx_cc_hook,
                                    partition_id_tensor)

    install_neuronx_cc_hook()
    pname = nc.partition_id_tensor.name if nc.partition_id_tensor else None
    in_names, out_names, out_avals, zero_outs = [], [], [], []
    for alloc in nc.m.functions[0].allocations:
        if not isinstance(alloc, mybir.__dict__["MemoryLocationSet"]):
            continue
        name = alloc.memorylocations[0].name
        if alloc.kind == "ExternalInput":
            if name != pname:
                in_names.append(name)
        elif alloc.kind == "ExternalOutput":
            out_names.append(name)
            shape = tuple(alloc.tensor_shape)
            dtype = mybir.dt.np(alloc.dtype)
            out_avals.append(jax.core.ShapedArray(shape, dtype))
            zero_outs.append(np.zeros(shape, dtype))
    n_params = len(in_names)
    n_outs = len(out_avals)
    all_in = in_names + out_names + ([pname] if pname else [])

    def _body(*args):
        ops = list(args)
        if pname is not None:
            ops.append(partition_id_tensor())
        outs = _bass_exec_p.bind(
            *ops, out_avals=tuple(out_avals), in_names=tuple(all_in),
            out_names=tuple(out_names), lowering_input_output_aliases=(),
            sim_require_finite=True, sim_require_nnan=True, nc=nc)
        return tuple(outs)

    devices = jax.devices()[dev_offset:dev_offset + n_group]
    mesh = Mesh(np.asarray(devices), ("core",))
    jf = jax.jit(shard_map(_body, mesh=mesh,
                           in_specs=(PartitionSpec("core"),) * (n_params + n_outs),
                           out_specs=(PartitionSpec("core"),) * n_outs,
                           check_rep=False), keep_unused=True)
    sh = jax.sharding.NamedSharding(mesh, PartitionSpec("core"))

    def run(in_maps):
        import jax as _jax
        concat = [np.concatenate([np.asarray(in_maps[c][nm]) for c in range(n_group)], axis=0)
                  for nm in in_names]
        concat += [np.concatenate([z] * n_group, axis=0) for z in zero_outs]
        dev_in = [_jax.device_put(a, sh) for a in concat]
        res = jf(*dev_in)
        return res, out_names

    return run


def kernel(h_atom, bias, w_ih_f, w_hh_f, b_ih_f, b_hh_f,
           w_ih_b, w_hh_b, b_ih_b, b_hh_b, batch, num_graphs, pad_len):
    import jax

    h_atom = np.asarray(h_atom, dtype=np.float32)
    batch_np = np.asarray(batch).astype(np.int64)
    n_atoms = h_atom.shape[0]

    counts = np.bincount(batch_np, minlength=G).astype(np.int64)
    start = np.concatenate([[0], np.cumsum(counts)[:-1]])
    pos = np.arange(n_atoms) - start[batch_np]

    # sort graphs by count desc, deal 4-way -> per-core rank lists with a
    # shared capacity profile
    order = np.argsort(-counts, kind="stable")
    ranks = [order[4 * np.arange(GPC) + c] for c in range(4)]  # graph ids per core mod
    caps = []
    for i in range(GPC):
        m = max(counts[order[4 * i + c]] for c in range(4))
        caps.append(int(max(8, ((m + 7) // 8) * 8)))
    caps[0] = min(caps[0], L)
    caps = [min(c, L) for c in caps]

    nf, nb, SP = _get_programs(caps)
    base = np.concatenate([[0], np.cumsum(caps)]).astype(int)

    def prep_core(core):
        d = "f" if core < 4 else "b"
        glist = ranks[core % 4]
        w_ih = np.asarray(w_ih_f if d == "f" else w_ih_b, dtype=np.float32)
        w_hh = np.asarray(w_hh_f if d == "f" else w_hh_b, dtype=np.float32)
        b_ih = np.asarray(b_ih_f if d == "f" else b_ih_b, dtype=np.float32)
        b_hh = np.asarray(b_hh_f if d == "f" else b_hh_b, dtype=np.float32)

        hc = np.full((SP, H), PAD_VAL, dtype=np.float32)  # cast to bf16 below
        for i, g in enumerate(glist):
            rows = h_atom[start[g]:start[g] + min(int(counts[g]), caps[i])]
            # bwd: reversed atoms end-aligned within the cap range, so slot q
            # lands at step q + L - cap_i, i.e. atom (orig pos l) at step L-1-l
            if d == "b":
                rows = rows[::-1]
                off = caps[i] - len(rows)
            else:
                off = 0
            hc[base[i] + off:base[i] + off + len(rows)] = rows

        pb = np.empty((1, H3), dtype=np.float32)
        pb[0, :2048] = b_ih[:2048] + b_hh[:2048]
        pb[0, 2048:] = b_ih[2048:]
        return {
            "h_cmp": hc.astype(_bf16),
            "w_ihT": np.ascontiguousarray(w_ih.T).reshape(KC, 128, H3)
                .astype(_bf16),
            "w_hhT": np.ascontiguousarray(w_hh.T).reshape(KC, 128, H3)
                .astype(_bf16),
            "biasT": np.ascontiguousarray(
                np.asarray(bias, dtype=np.float32).reshape(KC, 128).T),
            "pbias": pb.astype(_bf16),
            "padgr": np.tile(pb, (GPC, 1)).astype(_bf16),
            "bhh_n": b_hh[2048:].reshape(1, H).astype(_bf16),
            "ones_d": np.ones((1, 128), dtype=np.float32).astype(_bf16),
            "id64_d": np.eye(GPC, dtype=np.float32).astype(_bf16),
        }

    in_maps = [prep_core(c) for c in range(N_CORES)]
    global _LAST_IN_MAPS
    _LAST_IN_MAPS = in_maps

    rkey = ("runners",) + tuple(caps)
    if rkey not in _PROG_CACHE:
        _PROG_CACHE[rkey] = (_make_runner(nf, 4, 0), _make_runner(nb, 4, 4))
    run_f, run_b = _PROG_CACHE[rkey]

    res_f, names_f = run_f(in_maps[:4])
    res_b, names_b = run_b(in_maps[4:])
    jax.block_until_ready(res_f)
    jax.block_until_ready(res_b)

    stage_f = np.asarray(res_f[names_f.index("out_stage")]).astype(np.float32)
    stage_b = np.asarray(res_b[names_b.index("out_stage")]).astype(np.float32)

    out = np.empty((n_atoms, 2 * H), dtype=np.float32)
    for core in range(4):
        glist = ranks[core]
        inv = np.empty(G, dtype=np.int64)
        inv[glist] = np.arange(GPC)
        sel = np.isin(batch_np, glist)
        gi = inv[batch_np[sel]]
        p = np.minimum(pos[sel], L - 1)
        out[sel, :H] = stage_f[core * L + p, gi]
        out[sel, H:] = stage_b[core * L + (L - 1 - p), gi]
    return out

